# revision 15
# baseline (speedup 1.0000x reference)
"""Trainium2 Bass kernel for nn_ConditionalFusionModel (3x GATv2 + BN/ELU + pool + VAE head).

Self-contained: host-side sharding/preprocessing (numpy) + a single SPMD
Bass/Tile program on 8 NeuronCores, with AllGather/AllReduce collectives.

Key algorithmic points:
- Edges sorted by destination, sharded by graph-aligned node ranges; the
  chunk->block layout is normalized across cores so one SPMD program works.
- GATv2 logit: with u = t_c*(xl+xr)_c,  a_c*lrelu_0.2(v_c) = c1*u + c2*|u|.
  The linear term is folded into per-head "sum columns" of the weights; the
  per-edge cost is one fp16 matmul + one |.|-group tensor_reduce.
- No segment-max needed (logits bounded, exp safe); no denominator epsilon
  needed (self-loops guarantee den >= exp(logit) > 0).
- Aggregation = one-hot (edge->dst-slot) matmul of ex-weighted raw features;
  1/den normalization and the GAT linear transform are applied per node.
- BN stats are computed on pre-bias outputs (pad rows contribute zero) and
  AllReduced; bias/mean/var algebra is fixed up on device.
"""
import sys

sys.path.insert(0, "/opt/trn_rl_repo")

import numpy as np

N, E, G = 20000, 160000, 1000
F_IN, HID, LAT, H, MAXN = 11, 64, 128, 3, 29
D1, D2, D3 = 64, 128, 256
NCORE = 8
NP = 2560
NB = NP // 128
NN = NCORE * NP
EC = 128
GMAX = 128
OUTW = 2 * LAT + 2 + MAXN * F_IN + MAXN * MAXN  # 1418


# ------------------------------------------------------------------ host prep
def _build_shards(batch):
    starts = np.searchsorted(batch, np.arange(G))
    bounds = [0]
    for k in range(1, NCORE):
        target = N * k // NCORE
        gix = np.searchsorted(starts, target)
        cand = [starts[gix - 1] if gix > 0 else 0, starts[gix] if gix < G else N]
        bounds.append(int(min(cand, key=lambda s: abs(s - target))))
    bounds.append(N)
    bounds = np.array(bounds)
    assert (np.diff(bounds) <= NP).all()
    gb = batch[bounds[:-1]].astype(np.int64)
    gb = np.append(gb, G)
    assert (np.diff(gb) <= GMAX).all()
    return bounds, gb


def _prep_graph(inp):
    x = np.asarray(inp["x"], np.float32)
    ei = np.asarray(inp["edge_index"]).astype(np.int64)
    batch = np.asarray(inp["batch"]).astype(np.int64)
    loop = np.arange(N, dtype=np.int64)
    src = np.concatenate([ei[0], loop])
    dst = np.concatenate([ei[1], loop])
    bounds, gbounds = _build_shards(batch)

    core_of = np.searchsorted(bounds, np.arange(N), side="right") - 1
    local = np.arange(N) - bounds[core_of]
    padded_gid = (core_of * NP + local).astype(np.int64)

    order = np.argsort(dst, kind="stable")
    srcs, dsts = src[order], dst[order]

    # per (core, block) edge lists
    percb = [[None] * NB for _ in range(NCORE)]
    for k in range(NCORE):
        e0 = np.searchsorted(dsts, bounds[k])
        e1 = np.searchsorted(dsts, bounds[k + 1])
        es, ed = srcs[e0:e1], dsts[e0:e1]
        ldst = ed - bounds[k]
        for b in range(NB):
            m = (ldst >= b * 128) & (ldst < (b + 1) * 128)
            percb[k][b] = (padded_gid[es[m]], ldst[m] - b * 128)

    # common chunk->block map: per block, max chunks over cores
    cpb = [max(max((len(percb[k][b][0]) + EC - 1) // EC, 0) for k in range(NCORE))
           for b in range(NB)]
    cpb = [max(c, 1) for c in cpb]
    NCHUNK = sum(cpb)
    NCHUNK_PAD = int(np.ceil(NCHUNK / 8) * 8)
    cpb[NB - 1] += NCHUNK_PAD - NCHUNK
    NCHUNK = NCHUNK_PAD
    cblk = np.concatenate([np.full(cpb[b], b, np.int32) for b in range(NB)])
    cstart = np.cumsum([0] + cpb)

    cores = []
    for k in range(NCORE):
        csrc = np.zeros((NCHUNK, EC), np.int32)
        cslot = np.full((NCHUNK, EC), 127, np.int32)
        cmask = np.zeros((NCHUNK, EC), np.float32)
        for b in range(NB):
            bs, bslot = percb[k][b]
            ne = len(bs)
            c0 = cstart[b]
            full = cpb[b] * EC
            buf_s = np.zeros(full, np.int64)
            buf_t = np.full(full, 127, np.int64)
            buf_m = np.zeros(full, np.float32)
            buf_s[:ne] = bs
            buf_t[:ne] = bslot
            buf_m[:ne] = 1.0
            csrc[c0:c0 + cpb[b]] = buf_s.reshape(cpb[b], EC)
            cslot[c0:c0 + cpb[b]] = buf_t.reshape(cpb[b], EC)
            cmask[c0:c0 + cpb[b]] = buf_m.reshape(cpb[b], EC)
        S = np.zeros((NCHUNK, EC, 128), np.float16)
        S[np.arange(NCHUNK)[:, None], np.arange(EC)[None, :], cslot] = \
            cmask.astype(np.float16)
        n_k = int(bounds[k + 1] - bounds[k])
        nm = np.zeros(NP, np.float32)
        nm[:n_k] = 1.0
        b0, b1 = int(bounds[k]), int(bounds[k + 1])
        gslot = batch[b0:b1] - int(gbounds[k])
        pS = np.zeros((NB, 128, GMAX), np.float32)
        pS[np.arange(b1 - b0) // 128, np.arange(b1 - b0) % 128, gslot] = 1.0
        idx = csrc.reshape(-1)
        w = np.zeros((128, NCHUNK * EC // 16), np.int16)
        base = idx.reshape(-1, 16).T.astype(np.int16)
        for gg in range(8):
            w[gg * 16:(gg + 1) * 16, :] = base
        m3 = np.zeros((128, NCHUNK * 3), np.float32)
        for h in range(3):
            m3[:, h::3] = cmask.T
        xT = np.zeros((128, NP), np.float16)
        xT[:F_IN, :n_k] = x[b0:b1].T
        cores.append(dict(
            n_k=n_k, ngraph=int(gbounds[k + 1] - gbounds[k]),
            S=S, S_T=np.ascontiguousarray(S.transpose(0, 2, 1)),
            nodemask=nm, poolS=pS, idxw=w, mask3=m3, xTloc=xT,
            bounds=(b0, b1)))

    xtab = np.zeros((NN, 128), np.float16)
    for k in range(NCORE):
        b0, b1 = cores[k]["bounds"]
        xtab[k * NP:k * NP + (b1 - b0), :F_IN] = x[b0:b1]
    return cores, xtab, NCHUNK, cblk, cstart


def _prep_weights(inp):
    layers = []
    for (kl, kr, ka, kb, kg, kbe, din, dout) in [
            ("w1l", "w1r", "a1", "b1", "g1", "be1", F_IN, D1),
            ("w2l", "w2r", "a2", "b2", "g2", "be2", D1, D2),
            ("w3l", "w3r", "a3", "b3", "g3", "be3", D2, D3)]:
        wl = np.asarray(inp[kl], np.float32)
        wr = np.asarray(inp[kr], np.float32)
        a = np.asarray(inp[ka], np.float32)
        wl3 = wl.reshape(din, H, dout)
        wr3 = wr.reshape(din, H, dout)
        gsz = [int(((a[h] > 0) if s == 1 else (a[h] <= 0)).sum())
               for h in range(H) for s in (1, -1)]
        Cg = int(np.ceil(max(gsz) / 4) * 4)
        assert 3 * Cg + 3 <= 512
        WA_l = np.zeros((128, 1024), np.float32)
        WA_r = np.zeros((128, 1024), np.float32)
        gi = 0
        for h in range(H):
            for sgn in (1, -1):
                idx = np.where((a[h] > 0) if sgn == 1 else (a[h] <= 0))[0]
                t = a[h, idx] if sgn == 1 else 0.2 * a[h, idx]
                c1 = 0.6 if sgn == 1 else 3.0
                c2m = 0.4 if sgn == 1 else 2.0
                col0 = (gi % 3) * Cg + (512 if gi >= 3 else 0)
                WA_l[:din, col0:col0 + len(idx)] = wl3[:, h, idx] * (c2m * t)
                WA_r[:din, col0:col0 + len(idx)] = wr3[:, h, idx] * (c2m * t)
                sc = 512 + 3 * Cg + h
                WA_l[:din, sc] += (wl3[:, h, idx] * (c1 * t)).sum(1)
                WA_r[:din, sc] += (wr3[:, h, idx] * (c1 * t)).sum(1)
                gi += 1
        wlpad = np.zeros((128, H * dout), np.float32)
        for h in range(H):
            wlpad[:din, h * dout:(h + 1) * dout] = wl3[:, h]
        nslab = (dout + 127) // 128
        bn = np.zeros((128, 3 * nslab), np.float32)
        for arr, j in ((np.asarray(inp[kb], np.float32), 0),
                       (np.asarray(inp[kg], np.float32), 1),
                       (np.asarray(inp[kbe], np.float32), 2)):
            for s in range(nslab):
                seg = arr[s * 128:min((s + 1) * 128, dout)]
                bn[:len(seg), j * nslab + s] = seg
        layers.append(dict(WA_l=WA_l.astype(np.float16), WA_r=WA_r.astype(np.float16),
                           Cg=Cg, din=din, dout=dout, nslab=nslab,
                           wlpad=wlpad.astype(np.float16), bn=bn))
    return layers


def _prep_head(inp):
    f = lambda k: np.asarray(inp[k], np.float32)
    hw = {}
    hw["wmu"] = np.ascontiguousarray(
        f("wmu").reshape(2, 128, LAT).transpose(1, 0, 2).reshape(128, 2 * LAT))
    hw["wlv"] = np.ascontiguousarray(
        f("wlv").reshape(2, 128, LAT).transpose(1, 0, 2).reshape(128, 2 * LAT))
    hw["pw1"] = f("pw1")
    pw2 = np.zeros((128, 2), np.float32)
    pw2[:64] = f("pw2")
    hw["pw2"] = pw2
    dw0 = f("dw0")
    hw["dw0a"] = np.ascontiguousarray(dw0[:128])
    dw0b = np.zeros((128, 256), np.float32)
    dw0b[:2] = dw0[128:130]
    hw["dw0b"] = dw0b
    hw["dw1"] = np.ascontiguousarray(
        f("dw1").reshape(2, 128, 512).transpose(1, 0, 2).reshape(128, 2 * 512))
    hw["dw2"] = np.ascontiguousarray(
        f("dw2").reshape(4, 128, 1160).transpose(1, 0, 2).reshape(128, 4 * 1160))
    hb = np.zeros((128, 28), np.float32)
    hb[:, 0] = f("bmu")
    hb[:, 1] = f("blv")
    hb[:64, 2] = f("pb1")
    hb[:2, 3] = f("pb2")
    db0 = f("db0")
    hb[:, 4] = db0[:128]
    hb[:, 5] = db0[128:]
    db1, lng, lnb = f("db1"), f("lng"), f("lnb")
    for s in range(4):
        hb[:, 6 + s] = db1[s * 128:(s + 1) * 128]
        hb[:, 10 + s] = lng[s * 128:(s + 1) * 128]
        hb[:, 14 + s] = lnb[s * 128:(s + 1) * 128]
    db2 = f("db2")
    for s in range(10):
        seg = db2[s * 128:min((s + 1) * 128, 1160)]
        hb[:len(seg), 18 + s] = seg
    hw["hb"] = hb
    return hw


# ------------------------------------------------------------- device program
def _build(NCHUNK, cblk, cstart, layers_cfg):
    from concourse import bass, mybir, tile, bacc, library_config
    from concourse.masks import make_identity
    F32, F16, I16 = mybir.dt.float32, mybir.dt.float16, mybir.dt.int16
    AF = mybir.ActivationFunctionType
    ALU = mybir.AluOpType
    AX = mybir.AxisListType

    nc = bacc.Bacc("TRN2", target_bir_lowering=False, debug=False,
                   num_devices=NCORE, num_swdge_queues=4)
    din_ = lambda nm, sh, dt=F32: nc.dram_tensor(nm, sh, dt, kind="ExternalInput").ap()
    xtab = din_("xtab", [NN, 128], F16)
    idxw = din_("idxw", [128, NCHUNK * EC // 16], I16)
    smat = din_("smat", [NCHUNK * 128, 128], F16)
    stmat = din_("stmat", [NCHUNK * 128, 128], F16)
    mask3 = din_("mask3", [128, NCHUNK * 3])
    nminv = din_("nminv", [128, NB])
    nmask = din_("nmask", [128, NB])
    xTloc = din_("xTloc", [128, NP], F16)
    poolS = din_("poolS", [NB * 128, GMAX])
    WA_L = [din_(f"wa_l{i}", [128, 1024], F16) for i in range(3)]
    WA_R = [din_(f"wa_r{i}", [128, 1024], F16) for i in range(3)]
    WLH = [din_(f"wlh{i}", [128, H * layers_cfg[i]["dout"]], F16) for i in range(3)]
    BNW = [din_(f"bn{i}", [128, 3 * layers_cfg[i]["nslab"]]) for i in range(3)]
    HWD = {k: din_(k, sh) for k, sh in [
        ("wmu", [128, 256]), ("wlv", [128, 256]), ("pw1", [128, 64]),
        ("pw2", [128, 2]), ("dw0a", [128, 256]), ("dw0b", [128, 256]),
        ("dw1", [128, 1024]), ("dw2", [128, 4640]), ("hb", [128, 28])]}
    out_d = nc.dram_tensor("out", [GMAX, OUTW], F32, kind="ExternalOutput").ap()
    rg = [list(range(NCORE))]
    NG = NCHUNK // 8   # gather calls

    with tile.TileContext(nc) as tc:
        with tc.tile_pool(name="sb", bufs=1) as sb, \
             tc.tile_pool(name="sw2", bufs=2) as sw2, \
             tc.tile_pool(name="sw3", bufs=3) as sw3, \
             tc.tile_pool(name="psA", bufs=2, space="PSUM") as psA, \
             tc.tile_pool(name="psB", bufs=1, space="PSUM") as psB, \
             tc.tile_pool(name="dr", bufs=1, space="DRAM") as drp:
            nc.gpsimd.load_library(library_config.mlp)

            idn16 = sb.tile([128, 128], F16)
            make_identity(nc, idn16[:])
            idn32 = sb.tile([128, 128], F32)
            make_identity(nc, idn32[:])
            ones32 = sb.tile([128, 1], F32)
            nc.vector.memset(ones32[:], 1.0)
            eps5 = sb.tile([128, 1], F32)
            nc.vector.memset(eps5[:], 1e-5)

            idxw_sb = sb.tile([128, NCHUNK * EC // 16], I16)
            nc.sync.dma_start(out=idxw_sb[:], in_=idxw[:])
            mask3_sb = sb.tile([128, NCHUNK * 3], F32)
            nc.sync.dma_start(out=mask3_sb[:], in_=mask3[:])
            nminv_sb = sb.tile([128, NB], F32)
            nc.sync.dma_start(out=nminv_sb[:], in_=nminv[:])
            nmask_sb = sb.tile([128, NB], F32)
            nc.sync.dma_start(out=nmask_sb[:], in_=nmask[:])
            hb_sb = sb.tile([128, 28], F32)
            nc.sync.dma_start(out=hb_sb[:], in_=HWD["hb"][:])

            gt = [sb.tile([128, 8 * EC], F16, tag=f"gt{i}", name=f"gt{i}") for i in range(NG)]
            xrw_sb = sb.tile([128, NB * 1024], F16)
            hlocT = sb.tile([128, NP], F16)
            Abuf = sb.tile([128, NCHUNK * 6], F32)
            Scolb = sb.tile([128, NCHUNK * 3], F32)
            lgbuf = sb.tile([128, NCHUNK * 3], F32)
            exbuf = sb.tile([128, NCHUNK * 3], F32)
            hshard = sb.tile([128, NB * 256], F32)

            gin = [drp.tile([NP, 128], F16, tag=f"gin{i}", name=f"gin{i}") for i in range(2)]
            gout = [drp.tile([NN, 128], F16, tag=f"gout{i}", name=f"gout{i}") for i in range(2)]
            sti = [drp.tile([128, 8], F32, tag=f"sti{i}", name=f"sti{i}") for i in range(3)]
            sto = [drp.tile([128, 8], F32, tag=f"sto{i}", name=f"sto{i}") for i in range(3)]
            tabs = [xtab, gout[0].opt(), gout[1].opt()]

            for li in range(3):
                L = layers_cfg[li]
                din, dout, Cg, ns = L["din"], L["dout"], L["Cg"], L["nslab"]
                sc0 = 512 + 3 * Cg

                wal = sw2.tile([128, 1024], F16, tag="wal")
                nc.sync.dma_start(out=wal[:], in_=WA_L[li][:])
                war = sw2.tile([128, 1024], F16, tag="war")
                nc.sync.dma_start(out=war[:], in_=WA_R[li][:])
                wlh = sw2.tile([128, H * dout], F16, tag="wlh")
                nc.sync.dma_start(out=wlh[:], in_=WLH[li][:])
                bnw = sw2.tile([128, 3 * ns], F32, tag="bnw")
                nc.sync.dma_start(out=bnw[:], in_=BNW[li][:])

                # ---- A. local transposed features (for xr transform)
                if li == 0:
                    nc.sync.dma_start(out=hlocT[:], in_=xTloc[:])
                else:
                    pdout = layers_cfg[li - 1]["dout"]
                    if pdout < 128:
                        nc.gpsimd.memset(hlocT[:], 0)
                    for b in range(NB):
                        pt = psA.tile([128, 128], F32, space="PSUM", tag="t")
                        nc.tensor.transpose(out=pt[:pdout, :],
                                            in_=hshard[:, b * 256:b * 256 + pdout],
                                            identity=idn32[:])
                        if b % 2 == 0:
                            nc.vector.tensor_copy(
                                hlocT[:pdout, b * 128:(b + 1) * 128], pt[:pdout, :])
                        else:
                            nc.scalar.copy(
                                hlocT[:pdout, b * 128:(b + 1) * 128], pt[:pdout, :])

                # ---- B. xr_w = hlocT.T @ WA_r   [NP, 1024] fp16 (block-major)
                for b in range(NB):
                    pv = psA.tile([128, 1024], F32, space="PSUM", tag="v")
                    for j0 in (0, 512):
                        nc.tensor.matmul(pv[:, j0:j0 + 512],
                                         lhsT=hlocT[:, b * 128:(b + 1) * 128],
                                         rhs=war[:, j0:j0 + 512],
                                         start=True, stop=True)
                    if b % 2 == 0:
                        nc.vector.tensor_copy(xrw_sb[:, b * 1024:(b + 1) * 1024], pv[:])
                    else:
                        nc.scalar.copy(xrw_sb[:, b * 1024:(b + 1) * 1024], pv[:])

                # ---- C. row gathers
                for call in range(NG):
                    nc.gpsimd.dma_gather(
                        out_ap=gt[call][:].rearrange("p (c e) -> p c e", c=8),
                        in_ap=tabs[li][:],
                        idxs_ap=idxw_sb[:, call * 64:(call + 1) * 64],
                        num_idxs=1024, num_idxs_reg=1024, elem_size=128,
                        queue_num=call % 4)

                # ---- D. pass 1: per-chunk V matmuls + |.| reduce
                for c4 in range(NCHUNK // 4):
                    st4 = sw3.tile([128, 4 * 128], F16, tag="st4")
                    nc.sync.dma_start(
                        out=st4[:].rearrange("p (c e) -> p c e", c=4),
                        in_=stmat[c4 * 512:(c4 + 1) * 512, :].rearrange(
                            "(c p) e -> p c e", p=128))
                    for cc in range(4):
                        c = c4 * 4 + cc
                        blk = int(cblk[c])
                        gtile = gt[c // 8][:, (c % 8) * EC:(c % 8 + 1) * EC]
                        pt = psA.tile([128, 128], F16, space="PSUM", tag="t")
                        nc.tensor.transpose(out=pt[:], in_=gtile, identity=idn16[:])
                        hgT = sw3.tile([128, 128], F16, tag="hgT")
                        if c % 2 == 0:
                            nc.vector.tensor_copy(hgT[:], pt[:])
                        else:
                            nc.scalar.copy(hgT[:], pt[:])
                        pv = psA.tile([128, 1024], F32, space="PSUM", tag="v")
                        for j0 in (0, 512):
                            nc.tensor.matmul(pv[:, j0:j0 + 512], lhsT=hgT[:],
                                             rhs=wal[:, j0:j0 + 512],
                                             start=True, stop=False)
                            nc.tensor.matmul(pv[:, j0:j0 + 512],
                                             lhsT=st4[:, cc * 128:(cc + 1) * 128],
                                             rhs=xrw_sb[:, blk * 1024 + j0:
                                                        blk * 1024 + j0 + 512],
                                             start=False, stop=True)
                        nc.vector.tensor_reduce(
                            out=Abuf[:, c * 6:c * 6 + 3],
                            in_=pv[:, 0:3 * Cg].rearrange("p (g w) -> p g w", g=3),
                            op=ALU.add, axis=AX.X, apply_absolute_value=True)
                        nc.vector.tensor_reduce(
                            out=Abuf[:, c * 6 + 3:c * 6 + 6],
                            in_=pv[:, 512:512 + 3 * Cg].rearrange(
                                "p (g w) -> p g w", g=3),
                            op=ALU.add, axis=AX.X, apply_absolute_value=True)
                        nc.scalar.copy(Scolb[:, c * 3:c * 3 + 3],
                                       pv[:, sc0:sc0 + 3])

                # ---- E. batched logits -> masked ex
                Aview = Abuf[:].rearrange("p (c g t) -> p c g t", g=3, t=2)
                nc.vector.tensor_tensor(
                    out=lgbuf[:].rearrange("p (c g) -> p c g", g=3),
                    in0=Scolb[:].rearrange("p (c g) -> p c g", g=3),
                    in1=Aview[:, :, :, 0], op=ALU.add)
                nc.vector.tensor_tensor(
                    out=lgbuf[:].rearrange("p (c g) -> p c g", g=3),
                    in0=lgbuf[:].rearrange("p (c g) -> p c g", g=3),
                    in1=Aview[:, :, :, 1], op=ALU.subtract)
                nc.scalar.activation(exbuf[:], lgbuf[:], AF.Exp)
                nc.vector.tensor_tensor(out=exbuf[:], in0=exbuf[:],
                                        in1=mask3_sb[:], op=ALU.mult)

                # ---- F. pass 2: feeds + aggregation; node phase per block
                statacc = sw2.tile([128, 8], F32, tag="statacc")
                nc.vector.memset(statacc[:], 0.0)
                for c4 in range(NCHUNK // 4):
                    s4 = sw3.tile([128, 4 * 128], F16, tag="s4")
                    nc.sync.dma_start(
                        out=s4[:].rearrange("p (c e) -> p c e", c=4),
                        in_=smat[c4 * 512:(c4 + 1) * 512, :].rearrange(
                            "(c p) e -> p c e", p=128))
                    for cc in range(4):
                        c = c4 * 4 + cc
                        blk = int(cblk[c])
                        gtile = gt[c // 8][:, (c % 8) * EC:(c % 8 + 1) * EC]
                        feed = sw3.tile([128, 392], F16, tag="feed")
                        for h in range(3):
                            exc = exbuf[:, c * 3 + h:c * 3 + h + 1]
                            if h == 2:
                                nc.scalar.activation(feed[:, 256:384], gtile,
                                                     AF.Copy, bias=0.0, scale=exc)
                            else:
                                nc.vector.tensor_scalar(
                                    out=feed[:, h * 128:(h + 1) * 128], in0=gtile,
                                    scalar1=exc, scalar2=None, op0=ALU.mult)
                        nc.scalar.copy(feed[:, 384:387],
                                       exbuf[:, c * 3:c * 3 + 3])
                        first = c == int(cstart[blk])
                        last = c == int(cstart[blk + 1]) - 1
                        if first:
                            pagg = psB.tile([128, 512], F32, space="PSUM", tag="agg")
                            cur_agg = pagg
                        nc.tensor.matmul(cur_agg[:, 0:387], lhsT=s4[:, cc * 128:
                                                                    (cc + 1) * 128],
                                         rhs=feed[:, 0:387], start=first, stop=last)
                        if not last:
                            continue
                        # ------- node phase for block blk
                        b = blk
                        den = sw2.tile([128, 4], F32, tag="den")
                        nc.vector.tensor_scalar(
                            out=den[:, 0:3], in0=cur_agg[:, 384:387],
                            scalar1=nminv_sb[:, b:b + 1], scalar2=3.0,
                            op0=ALU.add, op1=ALU.mult)
                        rd = sw2.tile([128, 4], F32, tag="rd")
                        nc.vector.reciprocal(rd[:, 0:3], den[:, 0:3])
                        tsb = sw2.tile([128, 384], F16, tag="tsb")
                        for h in range(3):
                            nc.scalar.activation(
                                tsb[:, h * 128:(h + 1) * 128],
                                cur_agg[:, h * 128:(h + 1) * 128],
                                AF.Copy, bias=0.0, scale=rd[:, h:h + 1])
                        py = psB.tile([128, 256], F32, space="PSUM", tag="y")
                        for h in range(3):
                            pt2 = psA.tile([128, 128], F16, space="PSUM", tag="t")
                            nc.tensor.transpose(out=pt2[:],
                                                in_=tsb[:, h * 128:(h + 1) * 128],
                                                identity=idn16[:])
                            naT = sw3.tile([128, 128], F16, tag="naT")
                            if h % 2 == 0:
                                nc.vector.tensor_copy(naT[:], pt2[:])
                            else:
                                nc.scalar.copy(naT[:], pt2[:])
                            nc.tensor.matmul(py[:, 0:dout], lhsT=naT[:],
                                             rhs=wlh[:, h * dout:(h + 1) * dout],
                                             start=(h == 0), stop=(h == 2))
                        nc.scalar.copy(hshard[:, b * 256:b * 256 + dout],
                                       py[:, 0:dout])
                        sq = sw2.tile([128, 256], F32, tag="sq")
                        nc.scalar.square(sq[:, :dout], py[:, 0:dout])
                        pstt = psA.tile([128, 4], F32, space="PSUM", tag="t")
                        for s in range(ns):
                            w = min(128, dout - s * 128)
                            nc.tensor.matmul(
                                pstt[:w, s:s + 1],
                                lhsT=hshard[:, b * 256 + s * 128:b * 256 + s * 128 + w],
                                rhs=ones32[:], start=True, stop=True)
                            nc.tensor.matmul(
                                pstt[:w, ns + s:ns + s + 1],
                                lhsT=sq[:, s * 128:s * 128 + w],
                                rhs=ones32[:], start=True, stop=True)
                        nc.vector.tensor_tensor(out=statacc[:, 0:2 * ns],
                                                in0=statacc[:, 0:2 * ns],
                                                in1=pstt[:, 0:2 * ns], op=ALU.add)

                # ---- G. BN stats AllReduce + coefficients
                nc.sync.dma_start(out=sti[li].opt(), in_=statacc[:])
                nc.gpsimd.collective_compute(
                    "AllReduce", ALU.add, ins=[sti[li].opt()], outs=[sto[li].opt()],
                    replica_groups=rg)
                statr = sw2.tile([128, 8], F32, tag="statr")
                nc.sync.dma_start(out=statr[:], in_=sto[li].opt())

                cf = sw2.tile([128, 8 * 4], F32, tag="cf")  # scratch columns
                s1, q1, muv, tv, e2, vv, sdv, rv = [cf[:, 4 * i:4 * i + ns]
                                                    for i in range(8)]
                bcol = bnw[:, 0:ns]
                gcol = bnw[:, ns:2 * ns]
                becol = bnw[:, 2 * ns:3 * ns]
                nc.vector.tensor_scalar(out=s1, in0=statr[:, 0:ns], scalar1=1.0 / N,
                                        scalar2=None, op0=ALU.mult)
                nc.vector.tensor_scalar(out=q1, in0=statr[:, ns:2 * ns],
                                        scalar1=1.0 / N, scalar2=None, op0=ALU.mult)
                nc.vector.tensor_tensor(out=muv, in0=s1, in1=bcol, op=ALU.add)
                nc.vector.tensor_tensor(out=tv, in0=bcol, in1=s1, op=ALU.mult)
                nc.vector.tensor_scalar(out=tv, in0=tv, scalar1=2.0, scalar2=None,
                                        op0=ALU.mult)
                nc.vector.tensor_tensor(out=e2, in0=q1, in1=tv, op=ALU.add)
                nc.vector.tensor_tensor(out=tv, in0=bcol, in1=bcol, op=ALU.mult)
                nc.vector.tensor_tensor(out=e2, in0=e2, in1=tv, op=ALU.add)
                nc.vector.tensor_tensor(out=tv, in0=muv, in1=muv, op=ALU.mult)
                nc.vector.tensor_tensor(out=vv, in0=e2, in1=tv, op=ALU.subtract)
                nc.scalar.activation(sdv, vv, AF.Sqrt, bias=eps5[:, 0:1])
                nc.vector.reciprocal(rv, sdv)
                scb = sw2.tile([128, 2 * 4], F32, tag="scb")
                nc.vector.tensor_tensor(out=scb[:, 0:ns], in0=gcol, in1=rv,
                                        op=ALU.mult)
                nc.vector.tensor_tensor(out=tv, in0=bcol, in1=muv, op=ALU.subtract)
                nc.vector.tensor_tensor(out=tv, in0=scb[:, 0:ns], in1=tv,
                                        op=ALU.mult)
                nc.vector.tensor_tensor(out=scb[:, 4:4 + ns], in0=becol, in1=tv,
                                        op=ALU.add)
                srowf = sw2.tile([1, 1024], F32, tag="srowf")
                scbx = sb.tile([128, 1024], F32, tag="scbx", name="scbx")
                for j in list(range(ns)) + [4 + ss for ss in range(ns)]:
                    psc = psA.tile([128, 128], F32, space="PSUM", tag="t")
                    nc.tensor.transpose(out=psc[0:1, :], in_=scb[:, j:j + 1],
                                        identity=idn32[:])
                    nc.vector.tensor_copy(srowf[0:1, j * 128:(j + 1) * 128],
                                          psc[0:1, :])
                    nc.gpsimd.partition_broadcast(
                        scbx[:, j * 128:(j + 1) * 128],
                        srowf[0:1, j * 128:(j + 1) * 128], 128)

                # ---- H. BN apply + ELU + mask; write shard & AllGather input
                for b in range(NB):
                    for s in range(ns):
                        w = min(128, dout - s * 128)
                        ysl = hshard[:, b * 256 + s * 128: b * 256 + s * 128 + w]
                        t1 = sw2.tile([128, 128], F32, tag="bn_t1")
                        nc.vector.tensor_tensor(
                            out=t1[:, :w], in0=ysl,
                            in1=scbx[:, s * 128:s * 128 + w], op=ALU.mult)
                        nc.vector.tensor_tensor(
                            out=t1[:, :w], in0=t1[:, :w],
                            in1=scbx[:, (4 + s) * 128:(4 + s) * 128 + w], op=ALU.add)
                        t2 = sw2.tile([128, 128], F32, tag="bn_t2")
                        nc.scalar.activation(t2[:, :w], t1[:, :w], AF.Exp)
                        nc.vector.tensor_scalar(out=t2[:, :w], in0=t2[:, :w],
                                                scalar1=1.0, scalar2=0.0,
                                                op0=ALU.subtract, op1=ALU.min)
                        nc.vector.tensor_tensor(out=t1[:, :w], in0=t1[:, :w],
                                                in1=t2[:, :w], op=ALU.max)
                        nc.vector.tensor_scalar(
                            out=hshard[:, b * 256 + s * 128: b * 256 + s * 128 + w],
                            in0=t1[:, :w], scalar1=nmask_sb[:, b:b + 1],
                            op0=ALU.mult, scalar2=None)
                    if li < 2:
                        g16 = sw2.tile([128, 128], F16, tag="g16")
                        if dout < 128:
                            nc.vector.memset(g16[:], 0.0)
                        nc.scalar.copy(g16[:, :dout],
                                       hshard[:, b * 256:b * 256 + dout])
                        nc.sync.dma_start(out=gin[li][b * 128:(b + 1) * 128, :],
                                          in_=g16[:])
                if li < 2:
                    nc.gpsimd.collective_compute(
                        "AllGather", ALU.bypass, ins=[gin[li].opt()],
                        outs=[gout[li].opt()], replica_groups=rg)

            # ================= pooling =================
            ppool = psB.tile([128, 256], F32, space="PSUM", tag="agg")
            for b in range(NB):
                pS = sw2.tile([128, GMAX], F32, tag="pS")
                nc.sync.dma_start(out=pS[:], in_=poolS[b * 128:(b + 1) * 128, :])
                nc.tensor.matmul(ppool[:], lhsT=pS[:],
                                 rhs=hshard[:, b * 256:(b + 1) * 256],
                                 start=(b == 0), stop=(b == NB - 1))
            pooled = sb.tile([128, 256], F32)
            nc.vector.tensor_copy(pooled[:], ppool[:])

            # ================= VAE head (feature-major layout) ============
            hw_sb = {}
            for k, ap in HWD.items():
                if k in ("hb", "dw2"):
                    continue
                t = sb.tile([128, ap.shape[1]], F32, tag=f"hw_{k}", name=f"hw_{k}")
                nc.sync.dma_start(out=t[:], in_=ap[:])
                hw_sb[k] = t
            pldT = sb.tile([128, 256], F32)
            for s in range(2):
                ptr = psA.tile([128, 128], F32, space="PSUM", tag="t")
                nc.tensor.transpose(out=ptr[:], in_=pooled[:, s * 128:(s + 1) * 128],
                                    identity=idn32[:])
                nc.vector.tensor_copy(pldT[:, s * 128:(s + 1) * 128], ptr[:])

            def mm_chain(out_sb_slice, lhs_list, rhs_list, bias_col, act=None,
                         mrange=128):
                pm = psA.tile([128, 128], F32, space="PSUM", tag="t")
                nsteps = len(lhs_list)
                for i, (lh, rh) in enumerate(zip(lhs_list, rhs_list)):
                    nc.tensor.matmul(pm[:mrange, :], lhsT=lh, rhs=rh,
                                     start=(i == 0), stop=(i == nsteps - 1))
                nc.vector.tensor_scalar(out=out_sb_slice, in0=pm[:mrange, :],
                                        scalar1=bias_col, scalar2=None, op0=ALU.add)
                if act is not None:
                    nc.scalar.activation(out_sb_slice, out_sb_slice, act)

            muT = sb.tile([128, 128], F32)
            mm_chain(muT[:], [hw_sb["wmu"][:, 0:128], hw_sb["wmu"][:, 128:256]],
                     [pldT[:, 0:128], pldT[:, 128:256]], hb_sb[:, 0:1])
            lvT = sb.tile([128, 128], F32)
            mm_chain(lvT[:], [hw_sb["wlv"][:, 0:128], hw_sb["wlv"][:, 128:256]],
                     [pldT[:, 0:128], pldT[:, 128:256]], hb_sb[:, 1:2])
            spT = sb.tile([128, 128], F32)
            mm_chain(spT[0:64, :], [hw_sb["pw1"][:]], [muT[:]], hb_sb[0:64, 2:3],
                     act=AF.Silu, mrange=64)
            pT = sb.tile([128, 128], F32)
            mm_chain(pT[0:2, :], [hw_sb["pw2"][0:64, :]], [spT[0:64, :]],
                     hb_sb[0:2, 3:4], mrange=2)
            hdT = sb.tile([128, 256], F32)
            for s in range(2):
                mm_chain(hdT[:, s * 128:(s + 1) * 128],
                         [hw_sb["dw0a"][:, s * 128:(s + 1) * 128],
                          hw_sb["dw0b"][0:2, s * 128:(s + 1) * 128]],
                         [muT[:], pT[0:2, :]], hb_sb[:, 4 + s:5 + s])
            h1T = sb.tile([128, 512], F32)
            for o in range(4):
                mm_chain(h1T[:, o * 128:(o + 1) * 128],
                         [hw_sb["dw1"][:, 0 * 512 + o * 128:0 * 512 + (o + 1) * 128],
                          hw_sb["dw1"][:, 1 * 512 + o * 128:1 * 512 + (o + 1) * 128]],
                         [hdT[:, 0:128], hdT[:, 128:256]], hb_sb[:, 6 + o:7 + o])
            # layernorm over features (partition dim)
            plm = psA.tile([32, 128], F32, space="PSUM", tag="t")
            plq = psA.tile([32, 128], F32, space="PSUM", tag="t")
            sqT = sb.tile([128, 512], F32, tag="sqT")
            for o in range(4):
                nc.scalar.square(sqT[:, o * 128:(o + 1) * 128],
                                 h1T[:, o * 128:(o + 1) * 128])
            for o in range(4):
                nc.tensor.matmul(plm[0:1, :], lhsT=ones32[:],
                                 rhs=h1T[:, o * 128:(o + 1) * 128],
                                 start=(o == 0), stop=(o == 3))
            for o in range(4):
                nc.tensor.matmul(plq[0:1, :], lhsT=ones32[:],
                                 rhs=sqT[:, o * 128:(o + 1) * 128],
                                 start=(o == 0), stop=(o == 3))
            lnrow = sb.tile([1, 512], F32)
            nc.vector.tensor_scalar(out=lnrow[0:1, 0:128], in0=plm[0:1, :],
                                    scalar1=1.0 / 512, scalar2=None, op0=ALU.mult)
            nc.vector.tensor_scalar(out=lnrow[0:1, 128:256], in0=plq[0:1, :],
                                    scalar1=1.0 / 512, scalar2=None, op0=ALU.mult)
            nc.vector.tensor_tensor(out=lnrow[0:1, 256:384], in0=lnrow[0:1, 0:128],
                                    in1=lnrow[0:1, 0:128], op=ALU.mult)
            nc.vector.tensor_tensor(out=lnrow[0:1, 256:384], in0=lnrow[0:1, 128:256],
                                    in1=lnrow[0:1, 256:384], op=ALU.subtract)
            nc.scalar.activation(lnrow[0:1, 256:384], lnrow[0:1, 256:384], AF.Sqrt,
                                 bias=eps5[0:1, 0:1])
            nc.vector.reciprocal(lnrow[0:1, 384:512], lnrow[0:1, 256:384])
            lnbx = scbx
            nc.gpsimd.partition_broadcast(lnbx[:, 0:128], lnrow[0:1, 0:128], 128)
            nc.gpsimd.partition_broadcast(lnbx[:, 128:256], lnrow[0:1, 384:512], 128)
            h1n = sb.tile([128, 512], F32)
            for o in range(4):
                sl = h1n[:, o * 128:(o + 1) * 128]
                nc.vector.tensor_tensor(
                    out=sl, in0=h1T[:, o * 128:(o + 1) * 128],
                    in1=lnbx[:, 0:128], op=ALU.subtract)
                nc.vector.tensor_tensor(
                    out=sl, in0=sl, in1=lnbx[:, 128:256], op=ALU.mult)
                nc.vector.tensor_scalar(out=sl, in0=sl,
                                        scalar1=hb_sb[:, 10 + o:11 + o],
                                        scalar2=hb_sb[:, 14 + o:15 + o],
                                        op0=ALU.mult, op1=ALU.add)
                nc.scalar.activation(sl, sl, AF.Silu)
            # final head matmul + output assembly
            final = sb.tile([128, OUTW], F32)
            for comp, src_t in (("mu", muT), ("lv", lvT)):
                ptr = psA.tile([128, 128], F32, space="PSUM", tag="t")
                nc.tensor.transpose(out=ptr[:], in_=src_t[:], identity=idn32[:])
                off = 0 if comp == "mu" else 128
                nc.vector.tensor_copy(final[:, off:off + 128], ptr[:])
            ptr = psA.tile([128, 128], F32, space="PSUM", tag="t")
            nc.tensor.transpose(out=ptr[:], in_=pT[:], identity=idn32[:])
            nc.vector.tensor_copy(final[:, 256:258], ptr[:, 0:2])
            for o10 in range(10):
                w = min(128, 1160 - o10 * 128)
                dw2t = sw2.tile([128, 4 * 128], F32, tag="dw2t")
                for ks in range(4):
                    nc.sync.dma_start(
                        out=dw2t[:, ks * 128:ks * 128 + w],
                        in_=HWD["dw2"][:, ks * 1160 + o10 * 128:
                                       ks * 1160 + o10 * 128 + w])
                po = psA.tile([128, 128], F32, space="PSUM", tag="t")
                for ks in range(4):
                    nc.tensor.matmul(
                        po[:w, :],
                        lhsT=dw2t[:, ks * 128:ks * 128 + w],
                        rhs=h1n[:, ks * 128:(ks + 1) * 128],
                        start=(ks == 0), stop=(ks == 3))
                oT = sw2.tile([128, 128], F32, tag="oT")
                nc.vector.tensor_scalar(out=oT[:w, :], in0=po[:w, :],
                                        scalar1=hb_sb[:w, 18 + o10:19 + o10],
                                        scalar2=None, op0=ALU.add)
                ptr2 = psA.tile([128, 128], F32, space="PSUM", tag="t")
                nc.tensor.transpose(out=ptr2[:], in_=oT[:], identity=idn32[:])
                nc.vector.tensor_copy(final[:, 258 + o10 * 128:258 + o10 * 128 + w],
                                      ptr2[:, 0:w])
            # symmetrize adjacency
            adj = final[:, 258 + MAXN * F_IN:OUTW]
            tmp = sb.tile([128, MAXN * MAXN], F32, tag="sqT")
            nc.vector.tensor_tensor(
                out=tmp[:].rearrange("p (i j) -> p i j", i=MAXN),
                in0=adj.rearrange("p (i j) -> p i j", i=MAXN),
                in1=adj.rearrange("p (i j) -> p j i", i=MAXN),
                op=ALU.add)
            nc.vector.tensor_scalar(out=adj, in0=tmp[:], scalar1=0.5,
                                    scalar2=None, op0=ALU.mult)
            nc.sync.dma_start(out=out_d[:], in_=final[:])

    nc.compile()
    return nc


# ----------------------------------------------------------------- entrypoint
def kernel(**inputs):
    from concourse import bass_utils
    cores, xtab, NCHUNK, cblk, cstart = _prep_graph(inputs)
    layers = _prep_weights(inputs)
    hw = _prep_head(inputs)

    nc = _build(NCHUNK, cblk, cstart,
                [{k: L[k] for k in ("din", "dout", "Cg", "nslab")} for L in layers])

    in_maps = []
    for k in range(NCORE):
        core = cores[k]
        m = dict(
            xtab=xtab,
            idxw=core["idxw"],
            smat=core["S"].reshape(NCHUNK * 128, 128),
            stmat=core["S_T"].reshape(NCHUNK * 128, 128),
            mask3=core["mask3"],
            nminv=(1.0 - core["nodemask"]).reshape(NB, 128).T.copy(),
            nmask=core["nodemask"].reshape(NB, 128).T.copy(),
            xTloc=core["xTloc"],
            poolS=core["poolS"].reshape(NB * 128, GMAX),
            **{f"wa_l{i}": layers[i]["WA_l"] for i in range(3)},
            **{f"wa_r{i}": layers[i]["WA_r"] for i in range(3)},
            **{f"wlh{i}": layers[i]["wlpad"] for i in range(3)},
            **{f"bn{i}": layers[i]["bn"] for i in range(3)},
            **hw,
        )
        in_maps.append(m)

    res = bass_utils.run_bass_kernel_spmd(nc, in_maps, core_ids=list(range(NCORE)))
    outs = []
    for k in range(NCORE):
        outs.append(res.results[k]["out"][:cores[k]["ngraph"]])
    return np.concatenate(outs, 0).astype(np.float32)
